# revision 24
# baseline (speedup 1.0000x reference)
"""Multi-head self-attention (B=8, T=2048, C=192, H=6, HS=32) on 8 TRN2 cores.

Sharding: data-parallel over batch — core i computes batch element i fully
on-chip (no collectives). Host pre-transposes x and packs weights so the
device does zero transposes.

Per-core pipeline (engine in parentheses):
  qT/kT [d, t] = W^T @ xT                   (PE; PSUM->SBUF copies on ACT)
  v_aug [s, (h|1)]                          (PE; DVE copies; ones col -> rowsum)
  S^T   [s, t] = kT_h^T @ qT_h              (PE, K=32, one f32 PSUM bank/head)
  P^T = exp(S/sqrt(HS)) per head tile:      exact exp on ACT, or Schraudolph
        bf16-bit affine on DVE (bits = trunc(S1*x + S2) as int16, bitcast
        to bf16)
  O[t, d|rowsum] += P^T_tile^T @ v_aug      (PE, free dim 33 per head)
  O' = O * recip(rowsum)                    (DVE recip + broadcast mul)
  O'^T via PE transpose (identity), DVE copies to SBUF
  y[t, c] = O'^T.T @ Wp + bias              (PE, K=192 in 2 chunks + bias row)
"""

import numpy as np
import ml_dtypes
from contextlib import ExitStack

import concourse.bass as bass
import concourse.tile as tile
from concourse import bacc, mybir
from concourse.bass_utils import run_bass_kernel_spmd

B, T, C = 8, 2048, 192
H, HS = 6, 32
P = 128
TCH = 512            # t-chunk width per head (pair tile = 2*TCH)
NT = T // TCH        # 4
NS = T // P          # 16 s-tiles
E1 = HS + 1          # 33: per-head AV free dim (32 d + rowsum)
SCALE = 1.0 / float(np.sqrt(HS))
BF16 = mybir.dt.bfloat16
F32 = mybir.dt.float32
I16 = mybir.dt.int16
Exp = mybir.ActivationFunctionType.Exp
Alu = mybir.AluOpType

# Schraudolph bf16-bits exp: bf16(trunc(S1*x + S2)) ~ exp(SCALE*x), |rel|<4%
S1 = float((128.0 / np.log(2.0)) * SCALE)
S2 = 16250.0

# exp engine per (si, head): 'a' = ACT exact exp, 'd' = DVE Schraudolph.
# Heads 0-2 exact on ACT (plus head 3 on 2 of 8 si); heads 3-5 approx on DVE.
def exp_pat(si, h):
    if h < 3:
        return "a"
    if h == 3 and (si % 8 == 3 or si % 16 == 6):
        return "a"
    return "d"

_CACHE = {}


def build_nc():
    nc = bacc.Bacc()
    xT = nc.declare_dram_parameter("xT", [C, T], BF16, isOutput=False)
    wq = nc.declare_dram_parameter("wq", [C, H * HS], BF16, isOutput=False)
    wk = nc.declare_dram_parameter("wk", [C, H * HS], BF16, isOutput=False)
    wv = nc.declare_dram_parameter("wv", [C, H * HS], BF16, isOutput=False)
    wp = nc.declare_dram_parameter("wp", [H * HS, C], BF16, isOutput=False)
    bp = nc.declare_dram_parameter("bp", [1, C], BF16, isOutput=False)
    ident = nc.declare_dram_parameter("ident", [P, P], BF16, isOutput=False)
    out = nc.declare_dram_parameter("out", [T, C], F32, isOutput=True)

    with tile.TileContext(nc) as tc, ExitStack() as ctx:
        singles = ctx.enter_context(tc.tile_pool(name="singles", bufs=1))
        qk_pool = ctx.enter_context(tc.tile_pool(name="qk", bufs=1))
        pt_pool = ctx.enter_context(tc.tile_pool(name="ptp", bufs=20))
        post_pool = ctx.enter_context(tc.tile_pool(name="post", bufs=5))
        ysb_pool = ctx.enter_context(tc.tile_pool(name="ysb", bufs=3))

        # ---------------- load inputs ----------------
        # startup-critical loads on SP (weights + first xT columns); the
        # rest rides the idle Pool queue so the first matmuls start early
        w_sb = {}
        for name, dram, eng in (("q", wq, nc.sync), ("k", wk, nc.scalar)):
            a = singles.tile([P, H * HS], BF16, name=f"w{name}a")
            eng.dma_start(a, dram[0:P, :])
            b = singles.tile([C - P, H * HS], BF16, name=f"w{name}b")
            eng.dma_start(b, dram[P:C, :])
            w_sb[name] = (a, b)
        xT_a = singles.tile([P, T], BF16)
        nc.sync.dma_start(xT_a[:, 0:TCH], xT[0:P, 0:TCH])
        xT_b = singles.tile([C - P, T], BF16)
        nc.sync.dma_start(xT_b[:, 0:TCH], xT[P:C, 0:TCH])
        nc.gpsimd.dma_start(xT_a[:, TCH:T], xT[0:P, TCH:T])
        nc.gpsimd.dma_start(xT_b[:, TCH:T], xT[P:C, TCH:T])
        wva = singles.tile([P, H * HS], BF16, name="wva")
        nc.gpsimd.dma_start(wva, wv[0:P, :])
        wvb = singles.tile([C - P, H * HS], BF16, name="wvb")
        nc.gpsimd.dma_start(wvb, wv[P:C, :])
        w_sb["v"] = (wva, wvb)

        wp_a = singles.tile([P, C], BF16, name="wpa")
        nc.gpsimd.dma_start(wp_a, wp[0:P, :])
        wp_b = singles.tile([H * HS - P, C], BF16, name="wpb")
        nc.gpsimd.dma_start(wp_b, wp[P:H * HS, :])
        bp_sb = singles.tile([1, C], BF16)
        nc.gpsimd.dma_start(bp_sb, bp[:, :])
        id_sb = singles.tile([P, P], BF16, name="idsb")
        nc.gpsimd.dma_start(id_sb, ident[:, :])
        ones1 = singles.tile([1, P], BF16)
        nc.gpsimd.memset(ones1, 1.0)

        # preload exp activation table off the critical path
        warm = singles.tile([1, P], BF16, name="warm")
        nc.scalar.activation(warm, ones1, Exp)

        # v_aug: [s, si*(h|1)] with ones in col 32 of each head group
        v_aug = singles.tile([P, NS * H * E1], BF16, name="vaug")
        nc.gpsimd.memset(v_aug, 1.0)
        va_r = v_aug.rearrange("p (s h e) -> p s h e", s=NS, h=H)

        # ---------------- qT/kT/v destinations ----------------
        qT_a = qk_pool.tile([P, T], BF16)       # heads 0..3, d-major
        qT_b = qk_pool.tile([C - P, T], BF16)   # heads 4,5
        kT_a = qk_pool.tile([P, T], BF16)
        kT_b = qk_pool.tile([C - P, T], BF16)

        def hsrc(h):
            if h < 4:
                return kT_a, qT_a, HS * h
            return kT_b, qT_b, HS * (h - 4)

        with (
            tc.tile_pool(name="pstA", bufs=2, space="PSUM") as pstA_pool,
            tc.tile_pool(name="pstD", bufs=1, space="PSUM") as pstD_pool,
            tc.tile_pool(name="pav", bufs=1, space="PSUM") as pav_pool,
            tc.tile_pool(name="py", bufs=1, space="PSUM") as py_pool,
        ):
            def st_tile(eng):
                if eng == "a":
                    return pstA_pool.tile([P, TCH], F32, name="stp", tag="stA")
                return pstD_pool.tile([P, TCH], F32, name="stp", tag="stD1")

            def proj_chunk(proj, dlo, c0, eng):
                """one [dsz, TCH] chunk of the qT/kT projection."""
                dsz = P if dlo == 0 else C - P
                dst = ((qT_a, qT_b) if proj == "q" else
                       (kT_a, kT_b))[0 if dlo == 0 else 1]
                wa, wb = w_sb[proj]
                ps = st_tile(eng)
                nc.tensor.matmul(ps[0:dsz, :], wa[:, dlo:dlo + dsz],
                                 xT_a[:, c0:c0 + TCH], start=True, stop=False)
                nc.tensor.matmul(ps[0:dsz, :], wb[:, dlo:dlo + dsz],
                                 xT_b[:, c0:c0 + TCH], start=False, stop=True)
                if eng == "a":
                    nc.scalar.copy(dst[0:dsz, c0:c0 + TCH], ps[0:dsz, :])
                else:
                    nc.vector.tensor_copy(dst[0:dsz, c0:c0 + TCH],
                                          ps[0:dsz, :])

            def v_chunk(si):
                """v_aug values for one s-tile, via the pstD rotation."""
                s0 = si * P
                wva, wvb = w_sb["v"]
                ps = st_tile("d")
                nc.tensor.matmul(ps[:, 0:H * HS], xT_a[:, s0:s0 + P], wva,
                                 start=True, stop=False)
                nc.tensor.matmul(ps[:, 0:H * HS], xT_b[:, s0:s0 + P], wvb,
                                 start=False, stop=True)
                ps_r = ps[:, 0:H * HS].rearrange("p (h d) -> p h d", h=H)
                nc.vector.tensor_copy(va_r[:, si, :, 0:HS], ps_r)

            recs = {}

            def post_norm(av, tt):
                """normalize one t-tile: O' = O * recip(rowsum)."""
                av_t = av[tt // 2].rearrange("p (u h e) -> p u h e", u=2, h=H)
                u = tt % 2
                if u == 0:  # one reciprocal covers both u-slots of the bank
                    rec = post_pool.tile([P, 2 * H], F32, name="rec", tag="rec")
                    nc.vector.reciprocal(
                        rec.rearrange("p (u h) -> p u h", u=2),
                        av_t[:, :, :, HS])
                    recs[tt // 2] = rec
                rec = recs[tt // 2].rearrange("p (u h) -> p u h", u=2)[:, u, :]
                onorm = post_pool.tile([P, H * HS], BF16,
                                       name="onorm", tag="onorm")
                on_r = onorm.rearrange("p (h e) -> p h e", h=H)
                nc.vector.tensor_tensor(
                    on_r, av_t[:, u, :, 0:HS],
                    rec.unsqueeze(2).to_broadcast([P, H, HS]),
                    Alu.mult)
                return onorm

            def post_proj(tc0, onorm, tt, eng=None):
                """transpose, project, store one t-tile."""
                copy = nc.scalar.copy if eng == "a" else nc.vector.tensor_copy
                ycopy = nc.vector.tensor_copy if eng == "a" else copy
                # proj psum bank doubles as transpose scratch: cols
                # [C, C+P) viewed as bf16 hold O'^T before the copy-out
                ps = py_pool.tile([P, C + P], F32, name="psy", tag="psy")
                tp = ps[:, C:C + P].bitcast(BF16)
                nc.tensor.transpose(tp[:, 0:P], onorm[:, 0:P], id_sb)
                nc.tensor.transpose(
                    tp[0:H * HS - P, P:2 * P], onorm[:, P:H * HS], id_sb)
                oT = post_pool.tile([P, 2 * P], BF16, name="oT", tag="oT")
                copy(oT[:, 0:P], tp[:, 0:P])
                copy(oT[0:H * HS - P, P:2 * P], tp[0:H * HS - P, P:2 * P])
                nc.tensor.matmul(ps[:, 0:C], ones1, bp_sb,
                                 start=True, stop=False)
                nc.tensor.matmul(ps[:, 0:C], oT[:, 0:P], wp_a,
                                 start=False, stop=False)
                nc.tensor.matmul(ps[:, 0:C], oT[0:H * HS - P, P:2 * P], wp_b,
                                 start=False, stop=True)
                ysb = ysb_pool.tile([P, C], F32, name="ysbt", tag="ysbt")
                ycopy(ysb, ps[:, 0:C])
                nc.sync.dma_start(out[tc0 + tt * P:tc0 + (tt + 1) * P, :],
                                  ysb)

            def emit_av(av, si, ptiles):
                for h in range(H):
                    for tt in range(NT):
                        av_t = av[tt // 2].rearrange(
                            "p (u h e) -> p u h e", u=2, h=H)
                        nc.tensor.matmul(
                            av_t[:, tt % 2, h, :],
                            ptiles[h][:, tt * P:(tt + 1) * P],
                            va_r[:, si, h, :],
                            start=(si == 0 and h == 0 and tt % 2 == 0),
                            stop=(si == NS - 1),
                            skip_group_check=True)

            # prologue: just the chunks the first QKTs need, split ACT/DVE
            proj_chunk("q", 0, 0, "a")
            proj_chunk("k", 0, 0, "d")
            proj_chunk("q", P, 0, "a")
            proj_chunk("k", P, 0, "d")

            # deferred projection chunks, keyed by (tc index, si)
            deferred = {}
            for i, c0 in enumerate((TCH, 2 * TCH, 3 * TCH)):
                deferred.setdefault((0, 3 * i), []).extend(
                    [("k", 0, c0, "a"), ("k", P, c0, "a")])
            for tci in range(NT - 1):
                deferred.setdefault((tci, 8), []).append(
                    ("q", 0, (tci + 1) * TCH, "a"))
                deferred.setdefault((tci, 10), []).append(
                    ("q", P, (tci + 1) * TCH, "a"))

            prev1 = prev2 = None  # (av, si, ptiles): AV lags two si behind
            pending = None  # (tc0, onorms) awaiting post_proj
            old = None      # previous tc's (tc0, av) awaiting flush+norm
            av = None
            for tci, tc0 in enumerate(range(0, T, TCH)):
                old_av = av
                av = [pav_pool.tile([P, 2 * H * E1], F32,
                                    name=f"av{i}", tag=f"av{i}")
                      for i in range(2)]
                for si in range(NS):
                    s0 = si * P
                    d_heads = [h for h in range(3, H) if exp_pat(si, h) == "d"]
                    ptiles = [None] * H
                    # DVE pair: two heads share a 2-bank tile + one exp op
                    stp2 = pstD_pool.tile([P, 2 * TCH], F32,
                                          name="stp2", tag="stD2")
                    for half, h in enumerate(d_heads[:2]):
                        kT_t, qT_t, pb = hsrc(h)
                        nc.tensor.matmul(
                            stp2[:, half * TCH:(half + 1) * TCH],
                            kT_t[pb:pb + HS, s0:s0 + P],
                            qT_t[pb:pb + HS, tc0:tc0 + TCH],
                            start=True, stop=True, tile_position=(pb, 0))
                    ptp2 = pt_pool.tile([P, 2 * TCH], BF16,
                                        name="ptp2", tag="ptp2")
                    nc.vector.tensor_scalar(
                        ptp2.bitcast(I16), stp2, S1, S2, Alu.mult, Alu.add)
                    ptiles[d_heads[0]] = ptp2[:, 0:TCH]
                    ptiles[d_heads[1]] = ptp2[:, TCH:2 * TCH]
                    rest = d_heads[2:] + [h for h in range(H)
                                          if exp_pat(si, h) == "a"]
                    for h in rest:
                        kT_t, qT_t, pb = hsrc(h)
                        is_act = exp_pat(si, h) == "a"
                        stp = st_tile("a" if is_act else "d")
                        nc.tensor.matmul(
                            stp, kT_t[pb:pb + HS, s0:s0 + P],
                            qT_t[pb:pb + HS, tc0:tc0 + TCH],
                            start=True, stop=True, tile_position=(pb, 0))
                        ptp = pt_pool.tile([P, TCH], BF16,
                                           name="ptp", tag="ptp")
                        if is_act:
                            nc.scalar.activation(ptp, stp, Exp, scale=SCALE)
                        else:
                            nc.vector.tensor_scalar(
                                ptp.bitcast(I16), stp, S1, S2,
                                Alu.mult, Alu.add)
                        ptiles[h] = ptp
                    for args in deferred.get((tci, si), ()):
                        proj_chunk(*args)
                    if tci == 0:
                        v_chunk(si)
                    if si == 0 and old_av is not None:
                        emit_av(old_av, NS - 2, prev2[2])  # flush prev tc
                        emit_av(old_av, NS - 1, prev1[2])
                        prev2 = prev1 = None
                        pending = (tc0 - TCH,
                                   [post_norm(old_av, t) for t in (0, 1)])
                    if si == 1 and pending is not None and len(pending[1]) == 2:
                        pending[1].extend(post_norm(old_av, t) for t in (2, 3))
                    if si % 4 == 2 and pending is not None:
                        post_proj(pending[0], pending[1][si // 4], si // 4)
                        if si // 4 == NT - 1:
                            pending = None
                    if prev2 is not None:
                        emit_av(av, prev2[1], prev2[2])
                    prev2 = prev1
                    prev1 = (av, si, ptiles)
            emit_av(av, NS - 2, prev2[2])
            emit_av(av, NS - 1, prev1[2])
            for tt in range(NT):
                post_proj(tc0, post_norm(av, tt), tt, eng="a")

    nc.compile()
    return nc


def _get_nc():
    if "nc" not in _CACHE:
        _CACHE["nc"] = build_nc()
    return _CACHE["nc"]


def make_in_maps(x, Wq, Wk, Wv, Wproj, bproj):
    bf = ml_dtypes.bfloat16
    x = np.asarray(x, np.float32)
    pack = lambda w: np.ascontiguousarray(
        np.transpose(np.asarray(w, np.float32), (1, 0, 2)).reshape(C, H * HS)
    ).astype(bf)
    wq, wk, wv = pack(Wq), pack(Wk), pack(Wv)
    wp = np.ascontiguousarray(
        np.asarray(Wproj, np.float32).reshape(H * HS, C)).astype(bf)
    bp = np.asarray(bproj, np.float32).reshape(1, C).astype(bf)
    ident = np.eye(P, dtype=np.float32).astype(bf)
    maps = []
    for i in range(B):
        xti = np.ascontiguousarray(x[i].T).astype(bf)
        maps.append({"xT": xti, "wq": wq, "wk": wk, "wv": wv,
                     "wp": wp, "bp": bp, "ident": ident})
    return maps


def run(inputs, trace=False, **kw):
    nc = _get_nc()
    in_maps = make_in_maps(**inputs)
    res = run_bass_kernel_spmd(nc, in_maps, core_ids=list(range(B)),
                               trace=trace, **kw)
    y = np.stack([np.asarray(res.results[i]["out"], np.float32)
                  for i in range(B)], axis=0)
    return y, res


def kernel(**inputs):
    y, _ = run(inputs, trace=False)
    return y


# revision 25
# speedup vs baseline: 1.0070x; 1.0070x over previous
"""Multi-head self-attention (B=8, T=2048, C=192, H=6, HS=32) on 8 TRN2 cores.

Sharding: data-parallel over batch — core i computes batch element i fully
on-chip (no collectives). Host pre-transposes x and packs weights so the
device does zero transposes.

Per-core pipeline (engine in parentheses):
  qT/kT [d, t] = W^T @ xT                   (PE; PSUM->SBUF copies on ACT)
  v_aug [s, (h|1)]                          (PE; DVE copies; ones col -> rowsum)
  S^T   [s, t] = kT_h^T @ qT_h              (PE, K=32, one f32 PSUM bank/head)
  P^T = exp(S/sqrt(HS)) per head tile:      exact exp on ACT, or Schraudolph
        bf16-bit affine on DVE (bits = trunc(S1*x + S2) as int16, bitcast
        to bf16)
  O[t, d|rowsum] += P^T_tile^T @ v_aug      (PE, free dim 33 per head)
  O' = O * recip(rowsum)                    (DVE recip + broadcast mul)
  O'^T via PE transpose (identity), DVE copies to SBUF
  y[t, c] = O'^T.T @ Wp + bias              (PE, K=192 in 2 chunks + bias row)
"""

import numpy as np
import ml_dtypes
from contextlib import ExitStack

import concourse.bass as bass
import concourse.tile as tile
from concourse import bacc, mybir
from concourse.bass_utils import run_bass_kernel_spmd

B, T, C = 8, 2048, 192
H, HS = 6, 32
P = 128
TCH = 512            # t-chunk width per head (pair tile = 2*TCH)
NT = T // TCH        # 4
NS = T // P          # 16 s-tiles
E1 = HS + 1          # 33: per-head AV free dim (32 d + rowsum)
SCALE = 1.0 / float(np.sqrt(HS))
BF16 = mybir.dt.bfloat16
F32 = mybir.dt.float32
I16 = mybir.dt.int16
Exp = mybir.ActivationFunctionType.Exp
Alu = mybir.AluOpType

# Schraudolph bf16-bits exp: bf16(trunc(S1*x + S2)) ~ exp(SCALE*x), |rel|<4%
S1 = float((128.0 / np.log(2.0)) * SCALE)
S2 = 16250.0

# exp engine per (si, head): 'a' = ACT exact exp, 'd' = DVE Schraudolph.
# Heads 0-2 exact on ACT (plus head 3 on 2 of 8 si); heads 3-5 approx on DVE.
def exp_pat(si, h):
    if h < 3:
        return "a"
    if h == 3 and si % 8 == 3:
        return "a"
    return "d"

_CACHE = {}


def build_nc():
    nc = bacc.Bacc()
    xT = nc.declare_dram_parameter("xT", [C, T], BF16, isOutput=False)
    wq = nc.declare_dram_parameter("wq", [C, H * HS], BF16, isOutput=False)
    wk = nc.declare_dram_parameter("wk", [C, H * HS], BF16, isOutput=False)
    wv = nc.declare_dram_parameter("wv", [C, H * HS], BF16, isOutput=False)
    wp = nc.declare_dram_parameter("wp", [H * HS, C], BF16, isOutput=False)
    bp = nc.declare_dram_parameter("bp", [1, C], BF16, isOutput=False)
    ident = nc.declare_dram_parameter("ident", [P, P], BF16, isOutput=False)
    out = nc.declare_dram_parameter("out", [T, C], F32, isOutput=True)

    with tile.TileContext(nc) as tc, ExitStack() as ctx:
        singles = ctx.enter_context(tc.tile_pool(name="singles", bufs=1))
        qk_pool = ctx.enter_context(tc.tile_pool(name="qk", bufs=1))
        pt_pool = ctx.enter_context(tc.tile_pool(name="ptp", bufs=20))
        post_pool = ctx.enter_context(tc.tile_pool(name="post", bufs=5))
        ysb_pool = ctx.enter_context(tc.tile_pool(name="ysb", bufs=3))

        # ---------------- load inputs ----------------
        # startup-critical loads on SP (weights + first xT columns); the
        # rest rides the idle Pool queue so the first matmuls start early
        w_sb = {}
        for name, dram, eng in (("q", wq, nc.sync), ("k", wk, nc.scalar)):
            a = singles.tile([P, H * HS], BF16, name=f"w{name}a")
            eng.dma_start(a, dram[0:P, :])
            b = singles.tile([C - P, H * HS], BF16, name=f"w{name}b")
            eng.dma_start(b, dram[P:C, :])
            w_sb[name] = (a, b)
        xT_a = singles.tile([P, T], BF16)
        nc.sync.dma_start(xT_a[:, 0:TCH], xT[0:P, 0:TCH])
        xT_b = singles.tile([C - P, T], BF16)
        nc.sync.dma_start(xT_b[:, 0:TCH], xT[P:C, 0:TCH])
        nc.gpsimd.dma_start(xT_a[:, TCH:T], xT[0:P, TCH:T])
        nc.gpsimd.dma_start(xT_b[:, TCH:T], xT[P:C, TCH:T])
        wva = singles.tile([P, H * HS], BF16, name="wva")
        nc.gpsimd.dma_start(wva, wv[0:P, :])
        wvb = singles.tile([C - P, H * HS], BF16, name="wvb")
        nc.gpsimd.dma_start(wvb, wv[P:C, :])
        w_sb["v"] = (wva, wvb)

        wp_a = singles.tile([P, C], BF16, name="wpa")
        nc.gpsimd.dma_start(wp_a, wp[0:P, :])
        wp_b = singles.tile([H * HS - P, C], BF16, name="wpb")
        nc.gpsimd.dma_start(wp_b, wp[P:H * HS, :])
        bp_sb = singles.tile([1, C], BF16)
        nc.gpsimd.dma_start(bp_sb, bp[:, :])
        id_sb = singles.tile([P, P], BF16, name="idsb")
        nc.gpsimd.dma_start(id_sb, ident[:, :])
        ones1 = singles.tile([1, P], BF16)
        nc.gpsimd.memset(ones1, 1.0)

        # preload exp activation table off the critical path
        warm = singles.tile([1, P], BF16, name="warm")
        nc.scalar.activation(warm, ones1, Exp)

        # v_aug: [s, si*(h|1)] with ones in col 32 of each head group
        v_aug = singles.tile([P, NS * H * E1], BF16, name="vaug")
        nc.gpsimd.memset(v_aug, 1.0)
        va_r = v_aug.rearrange("p (s h e) -> p s h e", s=NS, h=H)

        # ---------------- qT/kT/v destinations ----------------
        qT_a = qk_pool.tile([P, T], BF16)       # heads 0..3, d-major
        qT_b = qk_pool.tile([C - P, T], BF16)   # heads 4,5
        kT_a = qk_pool.tile([P, T], BF16)
        kT_b = qk_pool.tile([C - P, T], BF16)

        def hsrc(h):
            if h < 4:
                return kT_a, qT_a, HS * h
            return kT_b, qT_b, HS * (h - 4)

        with (
            tc.tile_pool(name="pstA", bufs=2, space="PSUM") as pstA_pool,
            tc.tile_pool(name="pstD", bufs=1, space="PSUM") as pstD_pool,
            tc.tile_pool(name="pav", bufs=1, space="PSUM") as pav_pool,
            tc.tile_pool(name="py", bufs=1, space="PSUM") as py_pool,
        ):
            def st_tile(eng):
                if eng == "a":
                    return pstA_pool.tile([P, TCH], F32, name="stp", tag="stA")
                return pstD_pool.tile([P, TCH], F32, name="stp", tag="stD1")

            def proj_chunk(proj, dlo, c0, eng):
                """one [dsz, TCH] chunk of the qT/kT projection."""
                dsz = P if dlo == 0 else C - P
                dst = ((qT_a, qT_b) if proj == "q" else
                       (kT_a, kT_b))[0 if dlo == 0 else 1]
                wa, wb = w_sb[proj]
                ps = st_tile(eng)
                nc.tensor.matmul(ps[0:dsz, :], wa[:, dlo:dlo + dsz],
                                 xT_a[:, c0:c0 + TCH], start=True, stop=False)
                nc.tensor.matmul(ps[0:dsz, :], wb[:, dlo:dlo + dsz],
                                 xT_b[:, c0:c0 + TCH], start=False, stop=True)
                if eng == "a":
                    nc.scalar.copy(dst[0:dsz, c0:c0 + TCH], ps[0:dsz, :])
                else:
                    nc.vector.tensor_copy(dst[0:dsz, c0:c0 + TCH],
                                          ps[0:dsz, :])

            def v_chunk(si):
                """v_aug values for one s-tile, via the pstD rotation."""
                s0 = si * P
                wva, wvb = w_sb["v"]
                ps = st_tile("d")
                nc.tensor.matmul(ps[:, 0:H * HS], xT_a[:, s0:s0 + P], wva,
                                 start=True, stop=False)
                nc.tensor.matmul(ps[:, 0:H * HS], xT_b[:, s0:s0 + P], wvb,
                                 start=False, stop=True)
                ps_r = ps[:, 0:H * HS].rearrange("p (h d) -> p h d", h=H)
                nc.vector.tensor_copy(va_r[:, si, :, 0:HS], ps_r)

            recs = {}

            def post_norm(av, tt):
                """normalize one t-tile: O' = O * recip(rowsum)."""
                av_t = av[tt // 2].rearrange("p (u h e) -> p u h e", u=2, h=H)
                u = tt % 2
                if u == 0:  # one reciprocal covers both u-slots of the bank
                    rec = post_pool.tile([P, 2 * H], F32, name="rec", tag="rec")
                    nc.vector.reciprocal(
                        rec.rearrange("p (u h) -> p u h", u=2),
                        av_t[:, :, :, HS])
                    recs[tt // 2] = rec
                rec = recs[tt // 2].rearrange("p (u h) -> p u h", u=2)[:, u, :]
                onorm = post_pool.tile([P, H * HS], BF16,
                                       name="onorm", tag="onorm")
                on_r = onorm.rearrange("p (h e) -> p h e", h=H)
                nc.vector.tensor_tensor(
                    on_r, av_t[:, u, :, 0:HS],
                    rec.unsqueeze(2).to_broadcast([P, H, HS]),
                    Alu.mult)
                return onorm

            def post_proj(tc0, onorm, tt, eng=None):
                """transpose, project, store one t-tile."""
                copy = nc.scalar.copy if eng == "a" else nc.vector.tensor_copy
                ycopy = nc.vector.tensor_copy if eng == "a" else copy
                # proj psum bank doubles as transpose scratch: cols
                # [C, C+P) viewed as bf16 hold O'^T before the copy-out
                ps = py_pool.tile([P, C + P], F32, name="psy", tag="psy")
                tp = ps[:, C:C + P].bitcast(BF16)
                nc.tensor.transpose(tp[:, 0:P], onorm[:, 0:P], id_sb)
                nc.tensor.transpose(
                    tp[0:H * HS - P, P:2 * P], onorm[:, P:H * HS], id_sb)
                oT = post_pool.tile([P, 2 * P], BF16, name="oT", tag="oT")
                copy(oT[:, 0:P], tp[:, 0:P])
                copy(oT[0:H * HS - P, P:2 * P], tp[0:H * HS - P, P:2 * P])
                nc.tensor.matmul(ps[:, 0:C], ones1, bp_sb,
                                 start=True, stop=False)
                nc.tensor.matmul(ps[:, 0:C], oT[:, 0:P], wp_a,
                                 start=False, stop=False)
                nc.tensor.matmul(ps[:, 0:C], oT[0:H * HS - P, P:2 * P], wp_b,
                                 start=False, stop=True)
                ysb = ysb_pool.tile([P, C], F32, name="ysbt", tag="ysbt")
                ycopy(ysb, ps[:, 0:C])
                nc.sync.dma_start(out[tc0 + tt * P:tc0 + (tt + 1) * P, :],
                                  ysb)

            def emit_av(av, si, ptiles):
                for h in range(H):
                    for tt in range(NT):
                        av_t = av[tt // 2].rearrange(
                            "p (u h e) -> p u h e", u=2, h=H)
                        nc.tensor.matmul(
                            av_t[:, tt % 2, h, :],
                            ptiles[h][:, tt * P:(tt + 1) * P],
                            va_r[:, si, h, :],
                            start=(si == 0 and h == 0 and tt % 2 == 0),
                            stop=(si == NS - 1),
                            skip_group_check=True)

            # prologue: just the chunks the first QKTs need, split ACT/DVE
            proj_chunk("q", 0, 0, "a")
            proj_chunk("k", 0, 0, "d")
            proj_chunk("q", P, 0, "a")
            proj_chunk("k", P, 0, "d")

            # deferred projection chunks, keyed by (tc index, si)
            deferred = {}
            for i, c0 in enumerate((TCH, 2 * TCH, 3 * TCH)):
                deferred.setdefault((0, 3 * i), []).extend(
                    [("k", 0, c0, "a"), ("k", P, c0, "a")])
            for tci in range(NT - 1):
                deferred.setdefault((tci, 8), []).append(
                    ("q", 0, (tci + 1) * TCH, "a"))
                deferred.setdefault((tci, 10), []).append(
                    ("q", P, (tci + 1) * TCH, "a"))

            prev1 = prev2 = None  # (av, si, ptiles): AV lags two si behind
            pending = None  # (tc0, onorms) awaiting post_proj
            old = None      # previous tc's (tc0, av) awaiting flush+norm
            av = None
            for tci, tc0 in enumerate(range(0, T, TCH)):
                old_av = av
                av = [pav_pool.tile([P, 2 * H * E1], F32,
                                    name=f"av{i}", tag=f"av{i}")
                      for i in range(2)]
                for si in range(NS):
                    s0 = si * P
                    d_heads = [h for h in range(3, H) if exp_pat(si, h) == "d"]
                    ptiles = [None] * H
                    # DVE pair: two heads share a 2-bank tile + one exp op
                    stp2 = pstD_pool.tile([P, 2 * TCH], F32,
                                          name="stp2", tag="stD2")
                    for half, h in enumerate(d_heads[:2]):
                        kT_t, qT_t, pb = hsrc(h)
                        nc.tensor.matmul(
                            stp2[:, half * TCH:(half + 1) * TCH],
                            kT_t[pb:pb + HS, s0:s0 + P],
                            qT_t[pb:pb + HS, tc0:tc0 + TCH],
                            start=True, stop=True, tile_position=(pb, 0))
                    ptp2 = pt_pool.tile([P, 2 * TCH], BF16,
                                        name="ptp2", tag="ptp2")
                    nc.vector.tensor_scalar(
                        ptp2.bitcast(I16), stp2, S1, S2, Alu.mult, Alu.add)
                    ptiles[d_heads[0]] = ptp2[:, 0:TCH]
                    ptiles[d_heads[1]] = ptp2[:, TCH:2 * TCH]
                    rest = d_heads[2:] + [h for h in range(H)
                                          if exp_pat(si, h) == "a"]
                    for h in rest:
                        kT_t, qT_t, pb = hsrc(h)
                        is_act = exp_pat(si, h) == "a"
                        stp = st_tile("a" if is_act else "d")
                        nc.tensor.matmul(
                            stp, kT_t[pb:pb + HS, s0:s0 + P],
                            qT_t[pb:pb + HS, tc0:tc0 + TCH],
                            start=True, stop=True, tile_position=(pb, 0))
                        ptp = pt_pool.tile([P, TCH], BF16,
                                           name="ptp", tag="ptp")
                        if is_act:
                            nc.scalar.activation(ptp, stp, Exp, scale=SCALE)
                        else:
                            nc.vector.tensor_scalar(
                                ptp.bitcast(I16), stp, S1, S2,
                                Alu.mult, Alu.add)
                        ptiles[h] = ptp
                    for args in deferred.get((tci, si), ()):
                        proj_chunk(*args)
                    if tci == 0:
                        v_chunk(si)
                    if si == 0 and old_av is not None:
                        emit_av(old_av, NS - 2, prev2[2])  # flush prev tc
                        emit_av(old_av, NS - 1, prev1[2])
                        prev2 = prev1 = None
                        pending = (tc0 - TCH,
                                   [post_norm(old_av, t) for t in (0, 1)])
                    if si == 1 and pending is not None and len(pending[1]) == 2:
                        pending[1].extend(post_norm(old_av, t) for t in (2, 3))
                    if si % 4 == 2 and pending is not None:
                        post_proj(pending[0], pending[1][si // 4], si // 4)
                        if si // 4 == NT - 1:
                            pending = None
                    if prev2 is not None:
                        emit_av(av, prev2[1], prev2[2])
                    prev2 = prev1
                    prev1 = (av, si, ptiles)
            emit_av(av, NS - 2, prev2[2])
            emit_av(av, NS - 1, prev1[2])
            for tt in range(NT):
                post_proj(tc0, post_norm(av, tt), tt, eng="a")

    nc.compile()
    return nc


def _get_nc():
    if "nc" not in _CACHE:
        _CACHE["nc"] = build_nc()
    return _CACHE["nc"]


def make_in_maps(x, Wq, Wk, Wv, Wproj, bproj):
    bf = ml_dtypes.bfloat16
    x = np.asarray(x, np.float32)
    pack = lambda w: np.ascontiguousarray(
        np.transpose(np.asarray(w, np.float32), (1, 0, 2)).reshape(C, H * HS)
    ).astype(bf)
    wq, wk, wv = pack(Wq), pack(Wk), pack(Wv)
    wp = np.ascontiguousarray(
        np.asarray(Wproj, np.float32).reshape(H * HS, C)).astype(bf)
    bp = np.asarray(bproj, np.float32).reshape(1, C).astype(bf)
    ident = np.eye(P, dtype=np.float32).astype(bf)
    maps = []
    for i in range(B):
        xti = np.ascontiguousarray(x[i].T).astype(bf)
        maps.append({"xT": xti, "wq": wq, "wk": wk, "wv": wv,
                     "wp": wp, "bp": bp, "ident": ident})
    return maps


def run(inputs, trace=False, **kw):
    nc = _get_nc()
    in_maps = make_in_maps(**inputs)
    res = run_bass_kernel_spmd(nc, in_maps, core_ids=list(range(B)),
                               trace=trace, **kw)
    y = np.stack([np.asarray(res.results[i]["out"], np.float32)
                  for i in range(B)], axis=0)
    return y, res


def kernel(**inputs):
    y, _ = run(inputs, trace=False)
    return y
